# revision 32
# baseline (speedup 1.0000x reference)
"""Causal self-attention (GQA + RoPE) on 8 Trainium2 NeuronCores.

Sharding: data-parallel over batch (2) x tensor-parallel over KV-head groups
(4).  Core c handles batch b=c//4 and KV group g=c%4 (Q heads 4g..4g+3).
Each core computes qkv^T in a transposed [d, t] layout, runs attention with
scores in S^T[k, q] orientation (no transposes needed), then computes a
PARTIAL output projection: contraction over only its own 4 heads' 512
feature dims but over all 2048 output columns.  The host sums the 4 group
partials per batch (the "all-reduce after c_proj" is folded into the host
unshard step), so there is no on-device collective at all and the PE stream
never waits for communication.  Matmuls run in fp32r for scores and bf16
elsewhere.
"""

import sys

if "/opt/trn_rl_repo" not in sys.path:
    sys.path.insert(0, "/opt/trn_rl_repo")

import numpy as np

# Problem constants (hardcoded per contract)
B, T, C = 2, 2048, 2048
H, KV, HD = 16, 4, 128
G = 4               # TP groups (KV heads) per batch
N_CORES = 8
N_HL = H // KV      # local Q heads per core = 4
ROPE_THETA = 10000.0
TC = 512            # free-dim chunk for moving operands
NT = T // TC        # 4 t-chunks
NCT = C // 128      # 16 contraction tiles
NKT = T // 128      # 16 key tiles
SCALE = float(1.0 / np.sqrt(HD))

_COMPILED = {}


def _build(masked: bool):
    import concourse.bacc as bacc
    import concourse.tile as tile
    import concourse.mybir as mybir

    f32 = mybir.dt.float32

    nc = bacc.Bacc("TRN2", target_bir_lowering=False, debug=False,
                   num_devices=N_CORES, num_swdge_queues=4)

    bf16 = mybir.dt.bfloat16
    # inputs come pre-swizzled as [partition, row-block, col] so each SBUF
    # chunk loads with a single strided DMA (HWDGE queue slots are scarce)
    xT = nc.dram_tensor("xT", [128, NCT, T], bf16, kind="ExternalInput").ap()
    wqkvT = nc.dram_tensor("wqkvT", [128, NCT, (N_HL + 2) * HD], bf16,
                           kind="ExternalInput").ap()
    # w_proj rows for this core's 4 heads, [c_loc, o] layout
    wpT = nc.dram_tensor("wpT", [128, N_HL, C], bf16,
                         kind="ExternalInput").ap()
    cosT = nc.dram_tensor("cosT", [HD, T], f32, kind="ExternalInput").ap()
    sinT = nc.dram_tensor("sinT", [HD, T], f32, kind="ExternalInput").ap()
    binmask = nc.dram_tensor("binmask", [128, G * TC], bf16,
                             kind="ExternalInput").ap()
    kmask = None
    if masked:
        kmask = nc.dram_tensor("kmask", [128, NKT], f32,
                               kind="ExternalInput").ap()
    out = nc.dram_tensor("out", [T, C], f32, kind="ExternalOutput").ap()

    with tile.TileContext(nc, pool_alloc_mode="queue") as tc, \
         nc.allow_low_precision(reason="fp32r intermediates are intentional"):
        _build_body(nc, tc, mybir, f32,
                    xT, wqkvT, wpT, cosT, sinT, binmask, kmask, out)
    nc.compile()
    return nc


def _build_body(nc, tc, mybir, f32,
                xT, wqkvT, wpT, cosT, sinT, binmask, kmask, out):
    bf16 = mybir.dt.bfloat16
    f32r = mybir.dt.float32r
    from contextlib import ExitStack
    from concourse.masks import make_identity

    AF = mybir.ActivationFunctionType
    NR = N_HL + 2  # 6 row-tiles of qkv^T (4 q heads, k, v)
    NQ = TC // 128  # 4 128-subtiles per chunk

    with ExitStack() as ctx:
        # ---- pools (all share the kernel lifetime; queue allocator) ----
        const = ctx.enter_context(tc.tile_pool(name="const", bufs=1))
        rqkv = ctx.enter_context(tc.tile_pool(name="rqkv", bufs=1))
        work = ctx.enter_context(tc.tile_pool(name="work", bufs=3))
        raw_pool = rope_tmp = norm_pool = work
        o_sb_pool = ctx.enter_context(tc.tile_pool(name="o_sb", bufs=2))
        pt_pool = ctx.enter_context(tc.tile_pool(name="pt_pool", bufs=6))
        # PSUM: 4 + 2 + 2 = 8 banks
        mm_psum = ctx.enter_context(
            tc.tile_pool(name="mm_psum", bufs=4, space="PSUM"))
        y_psum = ctx.enter_context(
            tc.tile_pool(name="y_psum", bufs=2, space="PSUM"))
        sum_psum = ctx.enter_context(
            tc.tile_pool(name="sum_psum", bufs=2, space="PSUM"))

        ident = const.tile([128, 128], f32)
        make_identity(nc, ident[:])
        ones_sq_f = const.tile([128, 128], f32)
        nc.vector.memset(ones_sq_f[:], 1.0)
        ones_sq = const.tile([128, 128], bf16)
        nc.vector.tensor_copy(ones_sq[:], ones_sq_f[:])
        # dummy matmuls fill the initial DMA wait so the PE clock (HAM) is
        # already unthrottled when the first real chain starts
        warm_ps = mm_psum.tile([128, 128], f32, tag="mm", name="warm")
        for _ in range(60):
            nc.tensor.matmul(warm_ps[:], ones_sq[:], ones_sq[:],
                             start=True, stop=True)
        mask_sb = const.tile([128, G * TC], bf16)
        kmask_sb = None
        if kmask is not None:
            kmask_sb = const.tile([128, NKT], f32)
            nc.scalar.dma_start(kmask_sb[:], kmask[:])

        # w_proj rows (this core's heads), resident: [128, 4, 2048]
        # (DMA issued after phase A so it doesn't delay cos/sin)
        wp = const.tile([128, N_HL, C], bf16, tag="wp", name="wp")

        # resident activations (rotated q/k in [d, t] layout, v in [t, d])
        rq = [rqkv.tile([HD, T], f32r, tag=f"rq{h}", name=f"rq{h}")
              for h in range(N_HL)]
        rk = rqkv.tile([HD, T], f32r, tag="rk")
        v_sb = [rqkv.tile([128, HD], bf16, tag=f"v{i}", name=f"v{i}")
                for i in range(NKT)]
        # normalized attention outputs y^T per (head, chunk), resident
        yt = [[rqkv.tile([HD, TC], bf16, tag=f"yt{h}_{j}", name=f"yt{h}_{j}")
               for j in range(NT)] for h in range(N_HL)]

        # ============ Phase A: qkv^T projection + RoPE + V transpose ============
        def phase_a(j, wq, xt_pool, cos_sb, sin_sb, after_xt=None,
                    xt_engines=None):
            # 4 quarter-chunk DMAs (4 row-blocks each) per 512-token chunk
            xt_q = []
            for qb in range(4):
                xt_t = xt_pool.tile([128, 4, TC], bf16, tag=f"xtq{qb}",
                                    name=f"xtq{qb}_{j}")
                eng = xt_engines[qb] if xt_engines is not None else nc.sync
                eng.dma_start(xt_t[:], xT[:, 4 * qb:4 * (qb + 1),
                                          TC * j:TC * (j + 1)])
                xt_q.append(xt_t)
                if after_xt is not None and qb == 3:
                    after_xt()
            # V row (r=5) first: its transposes then never stall the PE at
            # the end of the chunk (the eviction overlaps the q/k rows)
            for r in (N_HL + 1, *range(N_HL + 1)):
                ps = mm_psum.tile([128, TC], f32, tag="mm", name=f"qkv{r}_{j}")
                for ct in range(NCT):
                    nc.tensor.matmul(ps[:],
                                     wq[ct // 2][:, ct % 2,
                                                 128 * r:128 * (r + 1)],
                                     xt_q[ct // 4][:, ct % 4, :],
                                     start=(ct == 0), stop=(ct == NCT - 1))
                if r < N_HL + 1:
                    # RoPE straight off PSUM:
                    #   dst = ps*cos + rot_half(ps)*sin_signed
                    dst = (rq[r] if r < N_HL else rk)[:, TC * j:TC * (j + 1)]
                    cs = cos_sb[:, TC * j:TC * (j + 1)]
                    sn = sin_sb[:, TC * j:TC * (j + 1)]
                    t1 = rope_tmp.tile([128, TC], f32, tag="t1",
                                       name=f"t1_{r}_{j}")
                    nc.vector.tensor_mul(t1[:], ps[:], cs[:])
                    t2 = rope_tmp.tile([128, TC], f32, tag="t2",
                                       name=f"t2_{r}_{j}")
                    nc.vector.tensor_mul(t2[0:64, :], ps[64:128, :], sn[0:64, :])
                    nc.vector.tensor_mul(t2[64:128, :], ps[0:64, :], sn[64:128, :])
                    nc.vector.tensor_add(dst, t1[:], t2[:])
                else:
                    # V: evict then transpose v^T [d, t] -> v [t, d]
                    rt = raw_pool.tile([128, TC], f32, tag="raw",
                                       name=f"vraw{j}")
                    nc.vector.tensor_copy(rt[:], ps[:])
                    for q in range(NQ):
                        pt = mm_psum.tile([128, 128], f32, tag="mm",
                                          name=f"vt{j}_{q}")
                        nc.tensor.transpose(
                            pt[:], rt[:, 128 * q:128 * (q + 1)], ident[:])
                        nc.vector.tensor_copy(v_sb[j * NQ + q][:], pt[:])

        # ============ Phase B: attention for q-chunk j ============
        def phase_b(j):
            nkt = (j + 1) * NQ  # causal limit in 128-k tiles
            ps_y = {}
            ps_sum = {}
            pt = {}

            def emit_scores(h, kt):
                r = kt - NQ * j
                # causal trim: cols [0, 128r) of this (k-tile, q-chunk) block
                # are fully masked.  f32r moving must stay >= 256 wide.
                p0 = 128 * r if r > 0 else 0      # bf16 ops (exp/mask/sum/y)
                s0 = min(p0, TC - 256)            # scores matmul (f32r)
                ps_s = mm_psum.tile([128, TC], f32, tag="mm",
                                    name=f"s{h}_{j}_{kt}")
                nc.tensor.matmul(ps_s[:, s0:TC],
                                 rk[:, 128 * kt:128 * (kt + 1)],
                                 rq[h][:, TC * j + s0:TC * (j + 1)],
                                 start=True, stop=True)
                p = pt_pool.tile([128, TC], bf16, tag="pt",
                                 name=f"pt{h}_{j}_{kt}")
                nc.scalar.activation(p[:, p0:TC], ps_s[:, p0:TC],
                                     AF.Exp, scale=SCALE)
                if r >= 0:
                    nc.vector.tensor_mul(
                        p[:, p0:TC], p[:, p0:TC],
                        mask_sb[:, TC * r + p0:TC * (r + 1)])
                if kmask_sb is not None:
                    nc.vector.tensor_scalar_mul(
                        p[:, p0:TC], p[:, p0:TC], kmask_sb[:, kt:kt + 1])
                pt[(h, kt)] = (p, p0)

            for hp in range(N_HL // 2):
                pair = (2 * hp, 2 * hp + 1)
                units = [(h, kt) for kt in range(nkt) for h in pair]
                emit_scores(*units[0])
                if len(units) > 1:
                    emit_scores(*units[1])
                for idx, (h, kt) in enumerate(units):
                    if idx + 2 < len(units):
                        emit_scores(*units[idx + 2])
                    if kt == 0:
                        ps_y[h] = y_psum.tile([HD, TC], f32, tag="y",
                                              name=f"y{h}_{j}")
                        ps_sum[h] = sum_psum.tile([128, TC], f32, tag="sum",
                                                  name=f"sum{h}_{j}")
                    p, p0 = pt.pop((h, kt))
                    nc.tensor.matmul(ps_sum[h][:, p0:TC], ones_sq[:],
                                     p[:, p0:TC],
                                     start=(kt == 0), stop=(kt == nkt - 1))
                    nc.tensor.matmul(ps_y[h][:, p0:TC], v_sb[kt][:],
                                     p[:, p0:TC],
                                     start=(kt == 0), stop=(kt == nkt - 1))
                    if kt != nkt - 1:
                        continue
                    # normalize straight off PSUM into the resident yt tile
                    rsum = norm_pool.tile([HD, TC], f32, tag="rsum",
                                          name=f"rs{h}_{j}")
                    nc.vector.reciprocal_approx_fast(rsum[:], ps_sum[h][:HD, :])
                    nc.vector.tensor_mul(yt[h][j][:], ps_y[h][:], rsum[:])

        # ============ Phase D: partial output projection for t-chunk j ============
        def phase_d(j):
            for tt in range(NQ):
                # one wide staging tile -> a single fully-contiguous 1MB DMA
                ot = o_sb_pool.tile([128, C], f32, tag="ot",
                                    name=f"ot{j}_{tt}")
                for oc in range(NT):
                    ps = mm_psum.tile([128, TC], f32, tag="mm",
                                      name=f"o{j}_{tt}_{oc}")
                    for h in range(N_HL):
                        nc.tensor.matmul(
                            ps[:], yt[h][j][:, 128 * tt:128 * (tt + 1)],
                            wp[:, h, TC * oc:TC * (oc + 1)],
                            start=(h == 0), stop=(h == N_HL - 1))
                    nc.vector.tensor_copy(ot[:, TC * oc:TC * (oc + 1)], ps[:])
                # two half-row DMAs on separate queues (one engine sustains
                # only ~50GB/s; a full 1MB write would back up the ot bufs)
                rows = out[TC * j + 128 * tt:TC * j + 128 * (tt + 1), :]
                nc.gpsimd.dma_start(rows[:, 0:C // 2], ot[:, 0:C // 2])
                nc.sync.dma_start(rows[:, C // 2:C], ot[:, C // 2:C])

        with tc.tile_pool(name="wq_pool", bufs=1) as wq_pool, \
             tc.tile_pool(name="xt_pool", bufs=2) as xt_pool, \
             tc.tile_pool(name="cs_pool", bufs=1) as cs_pool:
            cos_sb = cs_pool.tile([HD, T], f32)
            sin_sb = cs_pool.tile([HD, T], f32)
            # qkv weights as 8 x 384KB loads, interleaved with the first
            # chunk's x tiles across all three queues in consume order
            # (each DMA engine sustains only ~50GB/s)
            wq = [wq_pool.tile([128, 2, NR * HD], bf16, tag=f"wqp{i}",
                               name=f"wqp{i}")
                  for i in range(8)]
            for i, eng in zip(range(8), (nc.scalar, nc.gpsimd) * 4):
                eng.dma_start(wq[i][:], wqkvT[:, 2 * i:2 * (i + 1), :])
            # cos/sin per-chunk loads ride behind the weight loads (first
            # needed when qkv row 0 of chunk 0 finishes, ~15us in)
            for j in range(NT):
                nc.scalar.dma_start(cos_sb[:, TC * j:TC * (j + 1)],
                                    cosT[:, TC * j:TC * (j + 1)])
                nc.gpsimd.dma_start(sin_sb[:, TC * j:TC * (j + 1)],
                                    sinT[:, TC * j:TC * (j + 1)])

            def _load_aux():
                nc.sync.dma_start(mask_sb[:], binmask[:])

            phase_a(0, wq, xt_pool, cos_sb, sin_sb, after_xt=_load_aux,
                    xt_engines=[nc.sync, nc.sync, nc.scalar, nc.gpsimd])
            for h in range(N_HL):
                nc.gpsimd.dma_start(wp[:, h, :], wpT[:, h, :])
            for j in range(1, NT):
                phase_a(j, wq, xt_pool, cos_sb, sin_sb)

        for j in range(NT):
            phase_b(j)
            phase_d(j)


def _rope_tables():
    inv_freq = 1.0 / (ROPE_THETA ** (np.arange(0, HD, 2, dtype=np.float32) / HD))
    pos = np.arange(T, dtype=np.float32)
    freqs = pos[:, None] * inv_freq[None, :]
    emb = np.concatenate([freqs, freqs], axis=-1)          # [T, HD]
    cos = np.ascontiguousarray(np.cos(emb).astype(np.float32).T)   # [HD, T]
    sin = np.ascontiguousarray(np.sin(emb).astype(np.float32).T)
    sin[:64, :] *= -1.0                                    # sign for rotate_half
    return cos, sin


def _binmask():
    kk = np.arange(128)[:, None]
    qq = np.arange(TC)[None, :]
    blocks = [(kk <= qq - 128 * r).astype(np.float32) for r in range(G)]
    return np.ascontiguousarray(np.concatenate(blocks, axis=1))  # [128, 4*512]


def _prepare_in_maps(x, attention_mask, w_qkv, w_proj, masked):
    import ml_dtypes
    bf = ml_dtypes.bfloat16
    cos, sin = _rope_tables()
    bm = _binmask().astype(bf)

    def swizzle(a):
        # [R*128, F] -> [128, R, F] so row-blocks load in one strided DMA
        r = a.shape[0] // 128
        return np.ascontiguousarray(
            a.reshape(r, 128, a.shape[1]).transpose(1, 0, 2))

    in_maps = []
    for c in range(N_CORES):
        b, g = divmod(c, G)
        xT_b = swizzle(x[b].T.astype(bf))
        rows_q = w_qkv[512 * g:512 * (g + 1)]
        rows_k = w_qkv[H * HD + HD * g:H * HD + HD * (g + 1)]
        rows_v = w_qkv[(H + KV) * HD + HD * g:(H + KV) * HD + HD * (g + 1)]
        wqkvT_g = swizzle(np.concatenate(
            [rows_q, rows_k, rows_v], axis=0).T.astype(bf))
        # w_proj rows (contraction dims) for this core's 4 heads: [c_loc, o]
        wpT_g = swizzle(w_proj[:, 512 * g:512 * (g + 1)].T.astype(bf))
        m = {"xT": xT_b,
             "wqkvT": wqkvT_g,
             "wpT": wpT_g,
             "cosT": cos, "sinT": sin, "binmask": bm}
        if masked:
            m["kmask"] = np.ascontiguousarray(
                attention_mask[b].reshape(NKT, 128).T)
        in_maps.append(m)
    return in_maps


def kernel(x, attention_mask, w_qkv, w_proj):
    from concourse.bass_utils import run_bass_kernel_spmd

    x = np.asarray(x, dtype=np.float32)
    attention_mask = np.asarray(attention_mask, dtype=np.float32)
    w_qkv = np.asarray(w_qkv, dtype=np.float32)
    w_proj = np.asarray(w_proj, dtype=np.float32)

    masked = not bool((attention_mask == 1.0).all())
    if masked:
        attention_mask = (attention_mask != 0.0).astype(np.float32)

    if masked not in _COMPILED:
        _COMPILED[masked] = _build(masked)
    nc = _COMPILED[masked]

    in_maps = _prepare_in_maps(x, attention_mask, w_qkv, w_proj, masked)

    trace = bool(globals().get("_TRACE", False))
    res = run_bass_kernel_spmd(nc, in_maps, core_ids=list(range(N_CORES)),
                               trace=trace)
    globals()["_LAST_RESULT"] = res

    # unshard: sum the 4 head-group partials per batch
    y = np.empty((B, T, C), dtype=np.float32)
    for b in range(B):
        acc = res.results[4 * b]["out"].astype(np.float32, copy=True)
        for g in range(1, G):
            acc += res.results[4 * b + g]["out"]
        y[b] = acc
    return y
